# revision 27
# baseline (speedup 1.0000x reference)
"""Trainium2 Bass kernel for LogHarmonicLowering.

out[b, k*C + c, j, t] = wv0[k,j] * x[b, c, j+d_k, t] + wv1[k,j] * x[b, c, j+d_k+1, t]

with zero padding past the frequency range. The bilinear shift per k has a
constant integer part d_k plus per-(k,j) float32 weights wv0/wv1 precomputed
on host with the exact float32 arithmetic of the reference's grid method.

Distribution: data-parallel over batch — 8 cores, one batch element each.

Per-core scheme (v7). Measured facts driving the design (axon-tunnelled
TRN2, reps-slope timing with shared device buffers):
  - The per-core DMA path sustains ~355 GB/s TOTAL, shared across queues
    and directions (stores-only on one queue: 237 us for 80 MiB; splitting
    across SP+ACT or adding Pool-SWDGE does not raise it; 32 KB-descriptor
    channel-major stores are SLOWER at ~200 GB/s). So minimizing moved
    bytes is the main lever: v5 read x twice (aligned + 1-row-shifted
    copies, 117 MiB); v6+ reads x once (101 MiB).
  - The 1-row shift (X1[i] = x[i+1]) is computed on the otherwise-idle PE
    with fp32 permutation matmuls into PSUM — measured BIT-EXACT on HW,
    ~756 ns per [128x128]@[128,512] matmul incl. weight reload.
  - Compute-engine APs require quadrant-aligned partition starts, so a
    partition-offset operand shift on DVE is illegal; fp32r fails walrus.

Layout per core: partition dim = frequency (f), tiles [128, G, T] f32.
  - ACT: all loads (2 x 1 MiB per group) AND the 8 per-group tmp products
    tmp = wv1_k * X1 (Copy-activation with per-partition scale, PSUM in),
    lagged one group behind its loads so PE has produced the PSUM.
  - PE, per group: psA[:,c] = S@XA[:,c] + E@XB[:,c] (X1 for the A half;
    E adds the boundary row x[128] into partition 127), psB[:,c] =
    S@XB[:,c] (row 256 lands as exact 0 — its weight is always 0 because
    y1=256 is out of range). One matmul per PSUM bank (= one channel).
  - DVE, per (group, k): ZA = wv0*XA + tA; ZB = wv0*XB + tB
    (scalar_tensor_tensor), consuming ACT's tmp tiles.
  - SP: all stores. Every DMA covers a full 128-partition range (partial-
    partition DMAs measured 4-10x slower). Z stores are batched 4-channel
    (3-channel at the k boundary) "f c t" DMAs at a -d row offset: the
    first d partitions hold exact zeros (weight table zeroes partitions
    < d), landing on the previous channel's zero tail. Only the first
    channel of each shift needs a partial store, and the last channel of
    each shift an explicit d-row zero tail.

reps>1 repeats the whole kernel body back-to-back inside one program
(same DRAM in/out, identical final state). test.py uses the slope of
T(reps) over several rep counts (shared device buffers, Theil-Sen) to
measure steady-state per-execution device time, cancelling the ~80 ms
constant axon-tunnel dispatch latency and per-program constants.
"""

import functools
from contextlib import ExitStack

import numpy as np

import concourse.bass as bass
import concourse.mybir as mybir
from concourse.bass_utils import run_bass_kernel_spmd

FK = 5
ANCHOR = 1
OUT_LOG = 12.0
IN_LOG = 1.0
RADIX = 2.0

B, C, F, T = 8, 32, 256, 512
N_CORES = 8


def _host_weights(Fr):
    """Per-(k, j) bilinear weights, float32 ops matching the jax reference."""
    np_shift = (np.arange(FK) + 1) / ANCHOR
    ls = OUT_LOG * np.log(IN_LOG * np_shift) / np.log(RADIX)
    ls -= ls[ANCHOR - 1]
    ls32 = ls.astype(np.float32)
    shift_px = ls32 * np.float32(Fr / (Fr - 1))
    y = np.arange(Fr, dtype=np.float32)[None, :] + shift_px[:, None]
    y0f = np.floor(y)
    w1 = y - y0f
    w0 = np.float32(1.0) - w1
    y0 = y0f.astype(np.int32)
    y1 = y0 + 1
    v0 = ((y0 >= 0) & (y0 < Fr)).astype(np.float32)
    v1 = ((y1 >= 0) & (y1 < Fr)).astype(np.float32)
    wv0 = w0 * v0
    wv1 = w1 * v1
    d = y0[:, 0]
    # the integer shift is constant along j (fractional parts never round
    # across an integer boundary in f32 for these shifts)
    assert (y0 == d[:, None] + np.arange(Fr, dtype=np.int32)[None, :]).all()
    return wv0, wv1, d


def build_nc(C=C, Fr=F, T=T, G=4, NBUF=4, ZBUF=5, reps=1, variant=None):
    """Raw-bass per-core program: x[C,Fr,T] -> out[FK*C,Fr,T].

    variant: None/'real', or the timing-only probe 'no_dve' (stores stream
    memset Z tiles; ACT tmps, DVE combines, PE and Pool idle; WRONG output).
    """
    no_dve = variant == "no_dve"
    wv0, wv1, dks = _host_weights(Fr)
    H = Fr // 2
    nG = C // G
    nGr = nG * reps
    f32 = mybir.dt.float32
    dmax = int(dks.max())

    ncols = 2 * (FK - 1) * 2

    def col(a, ki, t):
        return (a * (FK - 1) + ki) * 2 + t

    # weight table, one column per (wv0/wv1, k, half); in INPUT row space,
    # with partitions below the integer shift d zeroed (those partitions of
    # ZA then hold exact zeros -> reused as the previous channel's zero tail)
    wvtab = np.zeros((H, ncols), np.float32)
    idx = np.arange(H)
    for ki in range(FK - 1):
        k = ki + 1
        d = int(dks[k])
        for a, wv in ((0, wv0), (1, wv1)):
            av = np.where(idx >= d, wv[k, np.maximum(idx - d, 0)], np.float32(0))
            wvtab[:, col(a, ki, 0)] = av
            wvtab[:, col(a, ki, 1)] = wv[k, idx + H - d]

    # PE shift matrices (lhsT layout: out = lhsT.T @ rhs).
    # S: out[i] = rhs[i+1]  -> lhsT[i+1, i] = 1
    # E: out[127] = rhs[0]  -> lhsT[0, 127] = 1
    ST = np.zeros((H, H), np.float32)
    for i in range(H - 1):
        ST[i + 1, i] = 1.0
    ET = np.zeros((H, H), np.float32)
    ET[0, H - 1] = 1.0

    mult = mybir.AluOpType.mult
    add = mybir.AluOpType.add
    bypass = mybir.AluOpType.bypass

    nc = bass.Bass(trn_type="TRN2")
    x_h = nc.dram_tensor("x", [C, Fr, T], f32, kind="ExternalInput")
    out_h = nc.dram_tensor("out", [FK * C, Fr, T], f32, kind="ExternalOutput")
    wv_h = nc.inline_tensor(wvtab, name="wvtab")
    st_h = nc.inline_tensor(ST, name="shiftS")
    et_h = nc.inline_tensor(ET, name="shiftE")
    of = out_h.rearrange("c f t -> (c f) t")

    def dram(ap):
        return ap.rearrange("c f t -> f c t")

    def zwin(r0, nch):
        """[H, nch, T] view of out rows r0 .. r0+H per channel, nch channels
        spaced Fr rows apart, r0 given in flat (c f) row units."""
        return (of[r0:r0 + nch * Fr, :]
                .rearrange("(c f) t -> f c t", f=Fr)[0:H, :, :])

    def dve_after(vg, k):          # s_dve value once ZB(vg,k) done (memset=#1)
        return 8 * vg + 2 * k + 3

    # tp value once tmpA/tmpB of (vg, k) written (ACT engine)
    def tpA(vg, k):
        return 8 * vg + 2 * k + 1

    def tpB(vg, k):
        return 8 * vg + 2 * k + 2

    with ExitStack() as ctx:
        sb = lambda shape, name: ctx.enter_context(
            nc.sbuf_tensor(name, shape, f32))
        wvt = sb([H, ncols], "wvt")
        stw = sb([H, H], "stw")
        etw = sb([H, H], "etw")
        zeros = sb([dmax, G, T], "zeros")
        XAB = [sb([H, G, 2, T], f"XAB{s}") for s in range(NBUF)]
        XA = [xab[:, :, 0, :] for xab in XAB]   # x rows 0..127 view
        XB = [xab[:, :, 1, :] for xab in XAB]   # x rows 128..255 view
        tA = [sb([H, G, T], f"tA{s}") for s in range(2)]
        tB = [sb([H, G, T], f"tB{s}") for s in range(2)]
        ZA = [sb([H, G, T], f"ZA{s}") for s in range(ZBUF)]
        ZB = [sb([H, G, T], f"ZB{s}") for s in range(ZBUF)]
        psA = ctx.enter_context(nc.psum_tensor("psA", [H, G, T], f32))
        psB = ctx.enter_context(nc.psum_tensor("psB", [H, G, T], f32))
        sem = lambda name: ctx.enter_context(nc.semaphore(name))
        s_wv = sem("s_wv")
        s_ld = [sem(f"s_ld{s}") for s in range(NBUF)]
        s_xst = [sem(f"s_xst{s}") for s in range(NBUF)]
        s_zst = [sem(f"s_zst{s}") for s in range(ZBUF)]
        s_pzt = [sem(f"s_pzt{s}") for s in range(ZBUF)]
        NZR = 8
        s_zrs = [sem(f"s_zr{r}") for r in range(NZR)]
        s_tp = sem("s_tp")
        s_mm = sem("s_mm")
        s_dve = sem("s_dve")
        block = ctx.enter_context(nc.Block())

        class W:  # monotone wait elision per engine
            def __init__(self, e):
                self.e, self.seen = e, {}
            def __call__(self, sem_, v):
                if v > self.seen.get(id(sem_), 0):
                    self.e.wait_ge(sem_, v)
                    self.seen[id(sem_)] = v

        nzr = [0]
        # per-z-slot cumulative s_zst (SP) and s_pzt (Pool partials) totals:
        # [z][u] = value before use u of the slot
        zsum = {z: [0] for z in range(ZBUF)}
        pzsum = {z: [0] for z in range(ZBUF)}
        for i in range(4 * nGr):
            z = i % ZBUF
            g = (i // 4) % nG
            zsum[z].append(zsum[z][-1] + 16 * 2)
            pzsum[z].append(pzsum[z][-1] + (16 if g == 0 else 0))

        def issue_loads(e, w, vg):
            s = vg % NBUF
            g = vg % nG
            if vg >= NBUF:
                pg = vg - NBUF
                pu = pg // NBUF
                if not no_dve:
                    w(s_dve, dve_after(pg, FK - 2))   # DVE consumed X of pg
                    w(s_mm, 2 * (pg + 1))            # PE consumed X of pg
                w(s_xst[s], 16 * (pu + 1))           # k0 stores of pg done
            xg = x_h[g * G:(g + 1) * G, :, :]
            e.dma_start(out=XAB[s][:, :, :, :],
                        in_=xg.rearrange("c (h f) t -> f c h t", h=2)).then_inc(s_ld[s], 16)

        def issue_tmps(e, w, vg):
            for k in range(FK - 1):
                i = 4 * vg + k
                t = i % 2
                if i >= 2:   # WAR: tmp slot reused from pair i-2; DVE read it
                    w(s_dve, dve_after(*divmod(i - 2, 4)))
                w(s_mm, 2 * vg + 1)                  # psA of vg ready
                e.mul(tA[t][:, :, :], psA[:, :, :],
                      wvt[:, col(1, k, 0):col(1, k, 0) + 1]).then_inc(s_tp, 1)
                w(s_mm, 2 * vg + 2)                  # psB of vg ready
                e.mul(tB[t][:, :, :], psB[:, :, :],
                      wvt[:, col(1, k, 1):col(1, k, 1) + 1]).then_inc(s_tp, 1)

        @block.scalar
        def _(e):
            w = W(e)
            if no_dve:
                for vg in range(nGr):
                    issue_loads(e, w, vg)
                return
            w(s_wv, 48)
            for vg in range(nGr + 1):
                if vg < nGr:
                    issue_loads(e, w, vg)
                if vg >= 1:
                    issue_tmps(e, w, vg - 1)

        @block.tensor
        def _(e):
            if no_dve:
                return
            w = W(e)
            w(s_wv, 48)   # shift matrices loaded (wvtab + S + E)
            for vg in range(nGr):
                s = vg % NBUF
                u = vg // NBUF
                w(s_ld[s], 16 * (u + 1))          # XAB of vg landed
                # psA = S @ XA + E @ XB, one matmul per PSUM bank (channel)
                if vg >= 1:
                    w(s_tp, tpA(vg - 1, FK - 2))  # tA(k3) of vg-1 read psA
                for c in range(G):
                    e.matmul(psA[:, c, :], stw[:, :], XA[s][:, c, :],
                             start=True, stop=False)
                for c in range(G):
                    mm = e.matmul(psA[:, c, :], etw[:, :], XB[s][:, c, :],
                                  start=False, stop=True)
                    if c == G - 1:
                        mm.then_inc(s_mm, 1)      # -> 2*vg + 1
                # psB = S @ XB (row 256 term has zero weight always)
                if vg >= 1:
                    w(s_tp, tpB(vg - 1, FK - 2))  # tB(k3) of vg-1 read psB
                for c in range(G):
                    mm = e.matmul(psB[:, c, :], stw[:, :], XB[s][:, c, :],
                                  start=True, stop=True)
                    if c == G - 1:
                        mm.then_inc(s_mm, 1)      # -> 2*vg + 2

        @block.sync
        def _(e):
            w = W(e)
            e.dma_start(out=wvt[:, :], in_=wv_h[:, :]).then_inc(s_wv, 16)
            e.dma_start(out=stw[:, :], in_=st_h[:, :]).then_inc(s_wv, 16)
            e.dma_start(out=etw[:, :], in_=et_h[:, :]).then_inc(s_wv, 16)
            for vg in range(nGr):
                s = vg % NBUF
                u = vg // NBUF
                g = vg % nG
                og0 = out_h[g * G:(g + 1) * G, :, :]
                w(s_ld[s], 16 * (u + 1))
                w(s_xst[s], 16 * u)
                e.dma_start(out=og0.rearrange("c (h f) t -> f c h t", h=2),
                            in_=XAB[s][:, :, :, :]).then_inc(s_xst[s], 16)
                for k in range(FK - 1):
                    d = int(dks[k + 1])
                    i = 4 * vg + k
                    z = i % ZBUF
                    uz = i // ZBUF
                    c0 = (k + 1) * C + g * G
                    og = out_h[c0:c0 + G, :, :]
                    w(s_zst[z], zsum[z][uz])
                    if not no_dve:
                        w(s_dve, dve_after(vg, k) - 1)   # ZA ready
                    if g == 0:
                        # first channel of this shift goes via the Pool
                        # queue (partial store); channels 1..3 batched at
                        # -d offset (the d zero partitions land on the
                        # previous channel's zero tail)
                        e.dma_start(out=zwin((c0 + 1) * Fr - d, G - 1),
                                    in_=ZA[z][:, 1:G, :]).then_inc(s_zst[z], 16)
                    else:
                        e.dma_start(out=zwin(c0 * Fr - d, G),
                                    in_=ZA[z][:, 0:G, :]).then_inc(s_zst[z], 16)
                    if not no_dve:
                        w(s_dve, dve_after(vg, k))       # ZB ready
                    e.dma_start(out=dram(og[:, H - d:Fr - d, :]), in_=ZB[z][:, :, :]).then_inc(s_zst[z], 16)
            # drain every DMA sem before program end
            for z in range(ZBUF):
                w(s_zst[z], zsum[z][-1])
            for s in range(NBUF):
                uses = sum(1 for vg in range(nGr) if vg % NBUF == s)
                w(s_xst[s], 16 * uses)

        @block.gpsimd
        def _(e):
            # small awkward stores off the critical SP queue: the per-shift
            # first-channel partial store and the per-shift last-channel
            # zero tail (~1.1 MiB total per execution)
            if no_dve:
                return
            w = W(e)
            for vg in range(nGr):
                g = vg % nG
                if g != 0 and g != nG - 1:
                    continue
                for k in range(FK - 1):
                    d = int(dks[k + 1])
                    i = 4 * vg + k
                    z = i % ZBUF
                    uz = i // ZBUF
                    if g == 0:
                        c0 = (k + 1) * C
                        og = out_h[c0:c0 + G, :, :]
                        w(s_dve, dve_after(vg, k) - 1)   # ZA ready
                        e.dma_start(out=dram(og[0:1, 0:H - d, :]),
                                    in_=ZA[z][d:H, 0:1, :]).then_inc(s_pzt[z], 16)
                    if g == nG - 1:
                        w(s_dve, 1)                      # zeros memset done
                        mlast = (k + 2) * C - 1
                        rz = mlast * Fr + Fr - d
                        n = nzr[0]
                        if n >= NZR:
                            w(s_zrs[n % NZR], 16 * (n // NZR))
                        e.dma_start(out=of[rz:rz + d, :].rearrange("f (o t) -> f o t", o=1),
                                    in_=zeros[0:d, 0:1, :]).then_inc(s_zrs[n % NZR], 16)
                        nzr[0] += 1
            for r in range(NZR):
                uses = sum(1 for n in range(nzr[0]) if n % NZR == r)
                w(s_zrs[r], 16 * uses)
            for z in range(ZBUF):
                w(s_pzt[z], pzsum[z][-1])

        @block.vector
        def _(e):
            w = W(e)
            e.memset(zeros[:, :, :], 0.0).then_inc(s_dve, 1)
            if no_dve:
                for z in range(ZBUF):
                    e.memset(ZA[z][:, :, :], 0.0)
                    e.memset(ZB[z][:, :, :], 0.0)
                return
            w(s_wv, 48)   # all three DMAs (wvt/stw/etw share one sem)
            for vg in range(nGr):
                s = vg % NBUF
                for k in range(FK - 1):
                    i = 4 * vg + k
                    t = i % 2
                    z = i % ZBUF
                    uz = i // ZBUF
                    w(s_zst[z], zsum[z][uz])         # Z slot recycle (SP)
                    w(s_pzt[z], pzsum[z][uz])        # Z slot recycle (Pool)
                    w(s_tp, tpA(vg, k))              # tA written by ACT
                    e.scalar_tensor_tensor(
                        ZA[z][:, :, :], XA[s][:, :, :],
                        wvt[:, col(0, k, 0):col(0, k, 0) + 1],
                        tA[t][:, :, :], mult, add).then_inc(s_dve, 1)
                    w(s_tp, tpB(vg, k))              # tB written by ACT
                    e.scalar_tensor_tensor(
                        ZB[z][:, :, :], XB[s][:, :, :],
                        wvt[:, col(0, k, 1):col(0, k, 1) + 1],
                        tB[t][:, :, :], mult, add).then_inc(s_dve, 1)
    return nc


@functools.lru_cache(maxsize=2)
def _get_nc(reps=1):
    return build_nc(reps=reps)


def _run(x, trace=False):
    in_maps = [{"x": np.ascontiguousarray(x[b])} for b in range(B)]
    res = run_bass_kernel_spmd(_get_nc(), in_maps, core_ids=list(range(N_CORES)),
                               trace=trace)
    out = np.stack([r["out"] for r in res.results], axis=0)
    return out, res


def kernel(x):
    x = np.asarray(x)
    assert x.shape == (B, C, F, T), x.shape
    out, _ = _run(x)
    return out


# revision 28
# speedup vs baseline: 1.0064x; 1.0064x over previous
"""Trainium2 Bass kernel for LogHarmonicLowering.

out[b, k*C + c, j, t] = wv0[k,j] * x[b, c, j+d_k, t] + wv1[k,j] * x[b, c, j+d_k+1, t]

with zero padding past the frequency range. The bilinear shift per k has a
constant integer part d_k plus per-(k,j) float32 weights wv0/wv1 precomputed
on host with the exact float32 arithmetic of the reference's grid method.

Distribution: data-parallel over batch — 8 cores, one batch element each.

Per-core scheme (v7). Measured facts driving the design (axon-tunnelled
TRN2, reps-slope timing with shared device buffers):
  - The per-core DMA path sustains ~355 GB/s TOTAL, shared across queues
    and directions (stores-only on one queue: 237 us for 80 MiB; splitting
    across SP+ACT or adding Pool-SWDGE does not raise it; 32 KB-descriptor
    channel-major stores are SLOWER at ~200 GB/s). So minimizing moved
    bytes is the main lever: v5 read x twice (aligned + 1-row-shifted
    copies, 117 MiB); v6+ reads x once (101 MiB).
  - The 1-row shift (X1[i] = x[i+1]) is computed on the otherwise-idle PE
    with fp32 permutation matmuls into PSUM — measured BIT-EXACT on HW,
    ~756 ns per [128x128]@[128,512] matmul incl. weight reload.
  - Compute-engine APs require quadrant-aligned partition starts, so a
    partition-offset operand shift on DVE is illegal; fp32r fails walrus.

Layout per core: partition dim = frequency (f), tiles [128, G, T] f32.
  - ACT: all loads (2 x 1 MiB per group) AND the 8 per-group tmp products
    tmp = wv1_k * X1 (Copy-activation with per-partition scale, PSUM in),
    lagged one group behind its loads so PE has produced the PSUM.
  - PE, per group: psA[:,c] = S@XA[:,c] + E@XB[:,c] (X1 for the A half;
    E adds the boundary row x[128] into partition 127), psB[:,c] =
    S@XB[:,c] (row 256 lands as exact 0 — its weight is always 0 because
    y1=256 is out of range). One matmul per PSUM bank (= one channel).
  - DVE, per (group, k): ZA = wv0*XA + tA; ZB = wv0*XB + tB
    (scalar_tensor_tensor), consuming ACT's tmp tiles.
  - SP: all stores. Every DMA covers a full 128-partition range (partial-
    partition DMAs measured 4-10x slower). Z stores are batched 4-channel
    (3-channel at the k boundary) "f c t" DMAs at a -d row offset: the
    first d partitions hold exact zeros (weight table zeroes partitions
    < d), landing on the previous channel's zero tail. Only the first
    channel of each shift needs a partial store, and the last channel of
    each shift an explicit d-row zero tail.

reps>1 repeats the whole kernel body back-to-back inside one program
(same DRAM in/out, identical final state). test.py uses the slope of
T(reps) over several rep counts (shared device buffers, Theil-Sen) to
measure steady-state per-execution device time, cancelling the ~80 ms
constant axon-tunnel dispatch latency and per-program constants.
"""

import functools
from contextlib import ExitStack

import numpy as np

import concourse.bass as bass
import concourse.mybir as mybir
from concourse.bass_utils import run_bass_kernel_spmd

FK = 5
ANCHOR = 1
OUT_LOG = 12.0
IN_LOG = 1.0
RADIX = 2.0

B, C, F, T = 8, 32, 256, 512
N_CORES = 8


def _host_weights(Fr):
    """Per-(k, j) bilinear weights, float32 ops matching the jax reference."""
    np_shift = (np.arange(FK) + 1) / ANCHOR
    ls = OUT_LOG * np.log(IN_LOG * np_shift) / np.log(RADIX)
    ls -= ls[ANCHOR - 1]
    ls32 = ls.astype(np.float32)
    shift_px = ls32 * np.float32(Fr / (Fr - 1))
    y = np.arange(Fr, dtype=np.float32)[None, :] + shift_px[:, None]
    y0f = np.floor(y)
    w1 = y - y0f
    w0 = np.float32(1.0) - w1
    y0 = y0f.astype(np.int32)
    y1 = y0 + 1
    v0 = ((y0 >= 0) & (y0 < Fr)).astype(np.float32)
    v1 = ((y1 >= 0) & (y1 < Fr)).astype(np.float32)
    wv0 = w0 * v0
    wv1 = w1 * v1
    d = y0[:, 0]
    # the integer shift is constant along j (fractional parts never round
    # across an integer boundary in f32 for these shifts)
    assert (y0 == d[:, None] + np.arange(Fr, dtype=np.int32)[None, :]).all()
    return wv0, wv1, d


def build_nc(C=C, Fr=F, T=T, G=4, NBUF=3, ZBUF=5, reps=1, variant=None):
    """Raw-bass per-core program: x[C,Fr,T] -> out[FK*C,Fr,T].

    variant: None/'real', or the timing-only probe 'no_dve' (stores stream
    memset Z tiles; ACT tmps, DVE combines, PE and Pool idle; WRONG output).
    """
    no_dve = variant == "no_dve"
    wv0, wv1, dks = _host_weights(Fr)
    H = Fr // 2
    nG = C // G
    nGr = nG * reps
    f32 = mybir.dt.float32
    dmax = int(dks.max())

    ncols = 2 * (FK - 1) * 2

    def col(a, ki, t):
        return (a * (FK - 1) + ki) * 2 + t

    # weight table, one column per (wv0/wv1, k, half); in INPUT row space,
    # with partitions below the integer shift d zeroed (those partitions of
    # ZA then hold exact zeros -> reused as the previous channel's zero tail)
    wvtab = np.zeros((H, ncols), np.float32)
    idx = np.arange(H)
    for ki in range(FK - 1):
        k = ki + 1
        d = int(dks[k])
        for a, wv in ((0, wv0), (1, wv1)):
            av = np.where(idx >= d, wv[k, np.maximum(idx - d, 0)], np.float32(0))
            wvtab[:, col(a, ki, 0)] = av
            wvtab[:, col(a, ki, 1)] = wv[k, idx + H - d]

    # PE shift matrices (lhsT layout: out = lhsT.T @ rhs).
    # S: out[i] = rhs[i+1]  -> lhsT[i+1, i] = 1
    # E: out[127] = rhs[0]  -> lhsT[0, 127] = 1
    ST = np.zeros((H, H), np.float32)
    for i in range(H - 1):
        ST[i + 1, i] = 1.0
    ET = np.zeros((H, H), np.float32)
    ET[0, H - 1] = 1.0

    mult = mybir.AluOpType.mult
    add = mybir.AluOpType.add
    bypass = mybir.AluOpType.bypass

    nc = bass.Bass(trn_type="TRN2")
    x_h = nc.dram_tensor("x", [C, Fr, T], f32, kind="ExternalInput")
    out_h = nc.dram_tensor("out", [FK * C, Fr, T], f32, kind="ExternalOutput")
    wv_h = nc.inline_tensor(wvtab, name="wvtab")
    st_h = nc.inline_tensor(ST, name="shiftS")
    et_h = nc.inline_tensor(ET, name="shiftE")
    of = out_h.rearrange("c f t -> (c f) t")

    def dram(ap):
        return ap.rearrange("c f t -> f c t")

    def zwin(r0, nch):
        """[H, nch, T] view of out rows r0 .. r0+H per channel, nch channels
        spaced Fr rows apart, r0 given in flat (c f) row units."""
        return (of[r0:r0 + nch * Fr, :]
                .rearrange("(c f) t -> f c t", f=Fr)[0:H, :, :])

    def dve_after(vg, k):          # s_dve value once ZB(vg,k) done (memset=#1)
        return 8 * vg + 2 * k + 3

    # tp value once tmpA/tmpB of (vg, k) written (ACT engine)
    def tpA(vg, k):
        return 8 * vg + 2 * k + 1

    def tpB(vg, k):
        return 8 * vg + 2 * k + 2

    with ExitStack() as ctx:
        sb = lambda shape, name: ctx.enter_context(
            nc.sbuf_tensor(name, shape, f32))
        wvt = sb([H, ncols], "wvt")
        stw = sb([H, H], "stw")
        etw = sb([H, H], "etw")
        zeros = sb([dmax, G, T], "zeros")
        XAB = [sb([H, G, 2, T], f"XAB{s}") for s in range(NBUF)]
        XA = [xab[:, :, 0, :] for xab in XAB]   # x rows 0..127 view
        XB = [xab[:, :, 1, :] for xab in XAB]   # x rows 128..255 view
        tA = [sb([H, G, T], f"tA{s}") for s in range(2)]
        tB = [sb([H, G, T], f"tB{s}") for s in range(2)]
        ZA = [sb([H, G, T], f"ZA{s}") for s in range(ZBUF)]
        ZB = [sb([H, G, T], f"ZB{s}") for s in range(ZBUF)]
        psA = ctx.enter_context(nc.psum_tensor("psA", [H, G, T], f32))
        psB = ctx.enter_context(nc.psum_tensor("psB", [H, G, T], f32))
        sem = lambda name: ctx.enter_context(nc.semaphore(name))
        s_wv = sem("s_wv")
        s_ld = [sem(f"s_ld{s}") for s in range(NBUF)]
        s_xst = [sem(f"s_xst{s}") for s in range(NBUF)]
        s_zst = [sem(f"s_zst{s}") for s in range(ZBUF)]
        s_pzt = [sem(f"s_pzt{s}") for s in range(ZBUF)]
        NZR = 8
        s_zrs = [sem(f"s_zr{r}") for r in range(NZR)]
        s_tp = sem("s_tp")
        s_mm = sem("s_mm")
        s_dve = sem("s_dve")
        block = ctx.enter_context(nc.Block())

        class W:  # monotone wait elision per engine
            def __init__(self, e):
                self.e, self.seen = e, {}
            def __call__(self, sem_, v):
                if v > self.seen.get(id(sem_), 0):
                    self.e.wait_ge(sem_, v)
                    self.seen[id(sem_)] = v

        nzr = [0]
        # per-z-slot cumulative s_zst (SP) and s_pzt (Pool partials) totals:
        # [z][u] = value before use u of the slot
        zsum = {z: [0] for z in range(ZBUF)}
        pzsum = {z: [0] for z in range(ZBUF)}
        for i in range(4 * nGr):
            z = i % ZBUF
            g = (i // 4) % nG
            zsum[z].append(zsum[z][-1] + 16 * 2)
            pzsum[z].append(pzsum[z][-1] + (16 if g == 0 else 0))

        def issue_loads(e, w, vg):
            s = vg % NBUF
            g = vg % nG
            if vg >= NBUF:
                pg = vg - NBUF
                pu = pg // NBUF
                if not no_dve:
                    w(s_dve, dve_after(pg, FK - 2))   # DVE consumed X of pg
                    w(s_mm, 2 * (pg + 1))            # PE consumed X of pg
                w(s_xst[s], 16 * (pu + 1))           # k0 stores of pg done
            xg = x_h[g * G:(g + 1) * G, :, :]
            e.dma_start(out=XAB[s][:, :, :, :],
                        in_=xg.rearrange("c (h f) t -> f c h t", h=2)).then_inc(s_ld[s], 16)

        def issue_tmps(e, w, vg):
            for k in range(FK - 1):
                i = 4 * vg + k
                t = i % 2
                if i >= 2:   # WAR: tmp slot reused from pair i-2; DVE read it
                    w(s_dve, dve_after(*divmod(i - 2, 4)))
                w(s_mm, 2 * vg + 1)                  # psA of vg ready
                e.mul(tA[t][:, :, :], psA[:, :, :],
                      wvt[:, col(1, k, 0):col(1, k, 0) + 1]).then_inc(s_tp, 1)
                w(s_mm, 2 * vg + 2)                  # psB of vg ready
                e.mul(tB[t][:, :, :], psB[:, :, :],
                      wvt[:, col(1, k, 1):col(1, k, 1) + 1]).then_inc(s_tp, 1)

        @block.scalar
        def _(e):
            w = W(e)
            if no_dve:
                for vg in range(nGr):
                    issue_loads(e, w, vg)
                return
            w(s_wv, 48)
            for vg in range(nGr + 1):
                if vg < nGr:
                    issue_loads(e, w, vg)
                if vg >= 1:
                    issue_tmps(e, w, vg - 1)

        @block.tensor
        def _(e):
            if no_dve:
                return
            w = W(e)
            w(s_wv, 48)   # shift matrices loaded (wvtab + S + E)
            for vg in range(nGr):
                s = vg % NBUF
                u = vg // NBUF
                w(s_ld[s], 16 * (u + 1))          # XAB of vg landed
                # psA = S @ XA + E @ XB, one matmul per PSUM bank (channel)
                if vg >= 1:
                    w(s_tp, tpA(vg - 1, FK - 2))  # tA(k3) of vg-1 read psA
                for c in range(G):
                    e.matmul(psA[:, c, :], stw[:, :], XA[s][:, c, :],
                             start=True, stop=False)
                for c in range(G):
                    mm = e.matmul(psA[:, c, :], etw[:, :], XB[s][:, c, :],
                                  start=False, stop=True)
                    if c == G - 1:
                        mm.then_inc(s_mm, 1)      # -> 2*vg + 1
                # psB = S @ XB (row 256 term has zero weight always)
                if vg >= 1:
                    w(s_tp, tpB(vg - 1, FK - 2))  # tB(k3) of vg-1 read psB
                for c in range(G):
                    mm = e.matmul(psB[:, c, :], stw[:, :], XB[s][:, c, :],
                                  start=True, stop=True)
                    if c == G - 1:
                        mm.then_inc(s_mm, 1)      # -> 2*vg + 2

        @block.sync
        def _(e):
            w = W(e)
            e.dma_start(out=wvt[:, :], in_=wv_h[:, :]).then_inc(s_wv, 16)
            e.dma_start(out=stw[:, :], in_=st_h[:, :]).then_inc(s_wv, 16)
            e.dma_start(out=etw[:, :], in_=et_h[:, :]).then_inc(s_wv, 16)
            for vg in range(nGr):
                s = vg % NBUF
                u = vg // NBUF
                g = vg % nG
                og0 = out_h[g * G:(g + 1) * G, :, :]
                w(s_ld[s], 16 * (u + 1))
                w(s_xst[s], 16 * u)
                e.dma_start(out=og0.rearrange("c (h f) t -> f c h t", h=2),
                            in_=XAB[s][:, :, :, :]).then_inc(s_xst[s], 16)
                for k in range(FK - 1):
                    d = int(dks[k + 1])
                    i = 4 * vg + k
                    z = i % ZBUF
                    uz = i // ZBUF
                    c0 = (k + 1) * C + g * G
                    og = out_h[c0:c0 + G, :, :]
                    w(s_zst[z], zsum[z][uz])
                    if not no_dve:
                        w(s_dve, dve_after(vg, k) - 1)   # ZA ready
                    if g == 0:
                        # first channel of this shift goes via the Pool
                        # queue (partial store); channels 1..3 batched at
                        # -d offset (the d zero partitions land on the
                        # previous channel's zero tail)
                        e.dma_start(out=zwin((c0 + 1) * Fr - d, G - 1),
                                    in_=ZA[z][:, 1:G, :]).then_inc(s_zst[z], 16)
                    else:
                        e.dma_start(out=zwin(c0 * Fr - d, G),
                                    in_=ZA[z][:, 0:G, :]).then_inc(s_zst[z], 16)
                    if not no_dve:
                        w(s_dve, dve_after(vg, k))       # ZB ready
                    e.dma_start(out=dram(og[:, H - d:Fr - d, :]), in_=ZB[z][:, :, :]).then_inc(s_zst[z], 16)
            # drain every DMA sem before program end
            for z in range(ZBUF):
                w(s_zst[z], zsum[z][-1])
            for s in range(NBUF):
                uses = sum(1 for vg in range(nGr) if vg % NBUF == s)
                w(s_xst[s], 16 * uses)

        @block.gpsimd
        def _(e):
            # small awkward stores off the critical SP queue: the per-shift
            # first-channel partial store and the per-shift last-channel
            # zero tail (~1.1 MiB total per execution)
            if no_dve:
                return
            w = W(e)
            for vg in range(nGr):
                g = vg % nG
                if g != 0 and g != nG - 1:
                    continue
                for k in range(FK - 1):
                    d = int(dks[k + 1])
                    i = 4 * vg + k
                    z = i % ZBUF
                    uz = i // ZBUF
                    if g == 0:
                        c0 = (k + 1) * C
                        og = out_h[c0:c0 + G, :, :]
                        w(s_dve, dve_after(vg, k) - 1)   # ZA ready
                        e.dma_start(out=dram(og[0:1, 0:H - d, :]),
                                    in_=ZA[z][d:H, 0:1, :]).then_inc(s_pzt[z], 16)
                    if g == nG - 1:
                        w(s_dve, 1)                      # zeros memset done
                        mlast = (k + 2) * C - 1
                        rz = mlast * Fr + Fr - d
                        n = nzr[0]
                        if n >= NZR:
                            w(s_zrs[n % NZR], 16 * (n // NZR))
                        e.dma_start(out=of[rz:rz + d, :].rearrange("f (o t) -> f o t", o=1),
                                    in_=zeros[0:d, 0:1, :]).then_inc(s_zrs[n % NZR], 16)
                        nzr[0] += 1
            for r in range(NZR):
                uses = sum(1 for n in range(nzr[0]) if n % NZR == r)
                w(s_zrs[r], 16 * uses)
            for z in range(ZBUF):
                w(s_pzt[z], pzsum[z][-1])

        @block.vector
        def _(e):
            w = W(e)
            e.memset(zeros[:, :, :], 0.0).then_inc(s_dve, 1)
            if no_dve:
                for z in range(ZBUF):
                    e.memset(ZA[z][:, :, :], 0.0)
                    e.memset(ZB[z][:, :, :], 0.0)
                return
            w(s_wv, 48)   # all three DMAs (wvt/stw/etw share one sem)
            for vg in range(nGr):
                s = vg % NBUF
                for k in range(FK - 1):
                    i = 4 * vg + k
                    t = i % 2
                    z = i % ZBUF
                    uz = i // ZBUF
                    w(s_zst[z], zsum[z][uz])         # Z slot recycle (SP)
                    w(s_pzt[z], pzsum[z][uz])        # Z slot recycle (Pool)
                    w(s_tp, tpA(vg, k))              # tA written by ACT
                    e.scalar_tensor_tensor(
                        ZA[z][:, :, :], XA[s][:, :, :],
                        wvt[:, col(0, k, 0):col(0, k, 0) + 1],
                        tA[t][:, :, :], mult, add).then_inc(s_dve, 1)
                    w(s_tp, tpB(vg, k))              # tB written by ACT
                    e.scalar_tensor_tensor(
                        ZB[z][:, :, :], XB[s][:, :, :],
                        wvt[:, col(0, k, 1):col(0, k, 1) + 1],
                        tB[t][:, :, :], mult, add).then_inc(s_dve, 1)
    return nc


@functools.lru_cache(maxsize=2)
def _get_nc(reps=1):
    return build_nc(reps=reps)


def _run(x, trace=False):
    in_maps = [{"x": np.ascontiguousarray(x[b])} for b in range(B)]
    res = run_bass_kernel_spmd(_get_nc(), in_maps, core_ids=list(range(N_CORES)),
                               trace=trace)
    out = np.stack([r["out"] for r in res.results], axis=0)
    return out, res


def kernel(x):
    x = np.asarray(x)
    assert x.shape == (B, C, F, T), x.shape
    out, _ = _run(x)
    return out
